# revision 45
# baseline (speedup 1.0000x reference)
"""Trainium2 Bass kernel for nn_Block_47098611368060 (dense transformer block).

v2: wall-clock-optimized. The dominant cost of this problem is shipping
inputs to the 8 axon-tunneled cores (~84 MB/s effective), so each unique
byte is shipped exactly once in bf16: weights are sharded 8-way row-wise
and AllGathered on device; per-batch activations are sharded across the
2 cores of a batch and pair-AllGathered.  LayerNorm scales are folded
into the following weight matrices on host; ln biases become per-feature
matmul biases applied for free by the ScalarE copy-out of QKV / an extra
row-vector add for proj.

Compute sharding (identical SPMD program on all 8 cores): core i =
(batch b=i//2, parity p=i%2) owns the interleaved query blocks
{2j+p : j=0..7} (128 rows each) of batch b and computes them end-to-end:
LN1 -> QKV -> causal attention -> proj -> residual -> LN2 -> MLP(gelu)
-> residual.  K/V are computed for the full 2048-row sequence per core.
Causal structure comes from a per-core additive tail mask (shipped data,
keeps the program parity-independent).
"""

import sys

for _p in ("/opt/trn_rl_repo",):
    if _p not in sys.path:
        sys.path.insert(0, _p)

import math
import numpy as np

try:
    import jax
    jax.config.update("jax_compilation_cache_dir", "/root/.jax_bass_cache")
    jax.config.update("jax_persistent_cache_min_compile_time_secs", 0.0)
    jax.config.update("jax_persistent_cache_min_entry_size_bytes", 0)
except Exception:
    pass

import concourse.bass as bass
import concourse.tile as tile
from concourse import bacc, mybir
from concourse.masks import make_identity

F32 = mybir.dt.float32
BF16 = mybir.dt.bfloat16

P = 128          # partitions
EPS = 1e-6
NEG = -1e9

# wblob column layout (bf16, per-rank 128-row shards)
WQKV0, WPROJ0, W10, W20 = 0, 3072, 4096, 8192
WCOLS = 12288


class Cfg:
    def __init__(self, S=2048, D=1024, NH=16, HD=64, HID=4096, NC=512):
        self.S, self.D, self.NH, self.HD, self.HID = S, D, NH, HD, HID
        self.NC = NC                  # moving-operand chunk
        self.SQ = S // 2              # own query rows per core
        self.RB = S // P              # seq row blocks
        self.QB = self.SQ // P        # own query blocks
        self.DB = D // P              # model-dim feature blocks
        self.HB = HID // P            # hidden feature blocks
        assert D % P == 0 and S % (2 * P) == 0 and HID % P == 0
        assert NH * HD == D and HD <= P
        assert NC >= 2 * P and self.SQ % NC == 0 and D % NC == 0 and S % NC == 0
        assert self.QB % 2 == 0


def _bcast(ap, parts, n):
    """[n] dram AP -> [parts, n] partition-broadcast AP."""
    return bass.AP(tensor=ap.tensor, offset=ap.offset, ap=[[0, parts]] + list(ap.ap))


# xg row-block position of seq block rb (xg = [even-parity blocks ; odd])
def xg_pos(rb):
    return (rb % 2) * 8 + rb // 2


# seq block held at yT position pos (yT columns follow xg order)
def pos_rb(pos):
    return 2 * (pos % 8) + pos // 8


def build(nc, tc, cfg, reps=1, stop_after=None):
    """Emit the full per-core program (identical on all 8 cores)."""
    import contextlib
    c = cfg
    NC = c.NC
    scale = 1.0 / math.sqrt(c.HD)
    DT = BF16

    U8 = mybir.dt.uint8
    U16 = mybir.dt.uint16

    # ---- I/O ----
    # bf16 payloads ship as byte planes (all high bytes, then all low bytes,
    # per row): the sign/exponent plane is low-entropy, so the axon wire
    # compression shrinks it.  Reassembled exactly on device.
    wblob = nc.dram_tensor("wblob", [P, 2 * WCOLS], U8, kind="ExternalInput").ap()
    xsh_u8 = nc.dram_tensor("xsh", [c.SQ, 2 * c.D], U8, kind="ExternalInput").ap()
    vcol = nc.dram_tensor("vcol", [P, 52], F32, kind="ExternalInput").ap()
    vrow = nc.dram_tensor("vrow", [2 * c.D], F32, kind="ExternalInput").ap()
    out = nc.dram_tensor("out", [c.SQ, c.D], DT, kind="ExternalOutput").ap()

    # ---- internal DRAM ----
    wb_b = nc.dram_tensor("wb_b", [P, WCOLS], DT).ap()
    wg = nc.dram_tensor("wg", [8 * P, WCOLS], DT, addr_space="Shared").ap()
    xsh = nc.dram_tensor("xsh_r", [c.SQ, c.D], DT).ap()
    xg = nc.dram_tensor("xg", [c.S, c.D], DT).ap()
    qT_s = nc.dram_tensor("qT_s", [c.D, c.SQ], DT).ap()
    kT_s = nc.dram_tensor("kT_s", [c.D, c.S], DT).ap()
    v_s = nc.dram_tensor("v_s", [c.S, c.D], DT).ap()

    # ---- reassemble byte planes, then gather ----
    with tc.tile_pool(name="replane", bufs=2) as rp:
        def replane(h_src, l_src, dst_bf, n):
            h8 = rp.tile([P, n], U8, tag="h8")
            nc.sync.dma_start(h8, h_src)
            l8 = rp.tile([P, n], U8, tag="l8")
            nc.sync.dma_start(l8, l_src)
            tmp = rp.tile([P, n], F32, tag="tmpf")
            nc.vector.scalar_tensor_tensor(
                tmp, h8, 256.0, l8,
                op0=mybir.AluOpType.mult, op1=mybir.AluOpType.add)
            u16t = rp.tile([P, n], U16, tag="u16t")
            nc.vector.tensor_copy(u16t, tmp)
            nc.sync.dma_start(dst_bf, u16t.bitcast(DT))

        HC = WCOLS // 4
        for q in range(4):
            replane(wblob[:, q * HC:(q + 1) * HC],
                    wblob[:, WCOLS + q * HC:WCOLS + (q + 1) * HC],
                    wb_b[:, q * HC:(q + 1) * HC], HC)
        xsh_u8b = xsh_u8.rearrange("(rb p) d -> rb p d", p=P)
        xsh_rb = xsh.rearrange("(rb p) d -> rb p d", p=P)
        for rb in range(c.QB):
            replane(xsh_u8b[rb][:, :c.D], xsh_u8b[rb][:, c.D:],
                    xsh_rb[rb], c.D)

    nc.gpsimd.collective_compute(
        "AllGather", mybir.AluOpType.bypass,
        replica_groups=[list(range(8))],
        ins=[wb_b.opt()], outs=[wg.opt()])
    nc.gpsimd.collective_compute(
        "AllGather", mybir.AluOpType.bypass,
        replica_groups=[[0, 1], [2, 3], [4, 5], [6, 7]],
        ins=[xsh.opt()], outs=[xg.opt()])

    # weight views into the gathered blob
    w_qkv = wg[:, WQKV0:WQKV0 + 3 * c.D]
    w_proj = wg[:, WPROJ0:WPROJ0 + c.D]
    w1 = wg[:, W10:W10 + c.HID]

    def w2_block(hb):  # [128, 1024] rows [128*hb : 128*(hb+1)] of w2
        r, j = hb // 4, hb % 4
        return wg[r * P:(r + 1) * P, W20 + j * c.D: W20 + (j + 1) * c.D]

    BN_FMAX = nc.vector.BN_STATS_FMAX
    BN_SD = nc.vector.BN_STATS_DIM
    BN_AD = nc.vector.BN_AGGR_DIM

    rep_loop = tc.For_i(0, reps, 1) if reps > 1 else contextlib.nullcontext()
    with rep_loop, tc.tile_pool(name="singles", bufs=1) as singles:
        ident = singles.tile([P, P], DT)
        make_identity(nc, ident)
        eps_t = singles.tile([P, 1], F32)
        nc.vector.memset(eps_t, EPS)
        vcol_sb = singles.tile([P, 52], F32)
        nc.sync.dma_start(vcol_sb, vcol)
        cq_s = vcol_sb[:, 0:8]      # per-feature q bias (pre-scaled)
        ck_b = vcol_sb[:, 8:16]     # per-feature k bias
        b1c = vcol_sb[:, 16:48]     # per-hidden-feature mlp bias
        thr = vcol_sb[:, 48:52]     # causal-mask thresholds (per parity)

        # mask_sb[:, i, f] = (f < thr[:, i]) * -1e9
        iota_f = singles.tile([P, 2 * P], F32)
        nc.gpsimd.iota(iota_f, pattern=[[1, 2 * P]], base=0,
                       channel_multiplier=0,
                       allow_small_or_imprecise_dtypes=True)
        mask_sb = singles.tile([P, 4, 2 * P], F32)
        for i in range(4):
            nc.vector.tensor_scalar(mask_sb[:, i, :], iota_f,
                                    thr[:, i:i + 1], NEG,
                                    op0=mybir.AluOpType.is_lt,
                                    op1=mybir.AluOpType.mult)

        beff_b = singles.tile([P, c.D], F32)
        nc.sync.dma_start(beff_b, _bcast(vrow[0:c.D], P, c.D))
        b2_b = singles.tile([P, c.D], F32)
        nc.sync.dma_start(b2_b, _bcast(vrow[c.D:2 * c.D], P, c.D))

        def layernorm(pool, x_t, y_t):
            """Row LN without scale/bias (folded into weights)."""
            sub = math.gcd(BN_FMAX, c.D)
            nsub = c.D // sub
            xg_ = x_t.rearrange("p (n s) -> p n s", s=sub)
            st = pool.tile([P, nsub, BN_SD], F32, tag="ln_st")
            for i in range(nsub):
                nc.vector.bn_stats(st[:, i, :], xg_[:, i, :])
            mv = pool.tile([P, BN_AD], F32, tag="ln_mv")
            nc.vector.bn_aggr(mv, st)
            std = pool.tile([P, 1], F32, tag="ln_std")
            nc.scalar.activation(std, mv[:, 1:2],
                                 mybir.ActivationFunctionType.Sqrt,
                                 bias=eps_t, scale=1.0)
            rstd = pool.tile([P, 1], F32, tag="ln_rstd")
            nc.vector.reciprocal(rstd, std)
            nc.vector.tensor_scalar(y_t, x_t, mv[:, 0:1], rstd,
                                    op0=mybir.AluOpType.subtract,
                                    op1=mybir.AluOpType.mult)

        out_b4 = out.rearrange("(rb p) (f q) -> rb p f q", p=P, q=P)

        def dump_and_stop(src3):  # src3: [P, DB, >=SQ] sbuf tile
            for rb in range(c.QB):
                nc.sync.dma_start(out_b4[rb],
                                  src3[:, :, rb * P:(rb + 1) * P])

        # ============ Phase A: LN1 + transpose ============
        with tc.tile_pool(name="yT_pool", bufs=1) as yT_pool:
            yT = yT_pool.tile([P, c.DB, c.S], DT)    # xg position order
            yTo = yT_pool.tile([P, c.DB, c.SQ], DT)  # own-j order
            with tc.tile_pool(name="ln_work", bufs=3) as lnw, \
                 tc.tile_pool(name="tp_ps", bufs=4, space="PSUM") as tp_ps:

                def ln_transpose(src_blocked, nblocks, dst):
                    for rb in range(nblocks):
                        x_t = lnw.tile([P, c.D], DT, tag="ln_x")
                        nc.sync.dma_start(x_t, src_blocked[rb])
                        y_t = lnw.tile([P, c.D], DT, tag="ln_y")
                        layernorm(lnw, x_t, y_t)
                        for f in range(c.DB):
                            pt = tp_ps.tile([P, P], DT, tag="tp")
                            nc.tensor.transpose(
                                pt, y_t[:, f * P:(f + 1) * P], ident)
                            nc.vector.tensor_copy(
                                dst[:, f, rb * P:(rb + 1) * P], pt)

                ln_transpose(xg.rearrange("(rb p) d -> rb p d", p=P), c.RB, yT)
                ln_transpose(xsh.rearrange("(rb p) d -> rb p d", p=P), c.QB, yTo)
            if stop_after == "A":
                dump_and_stop(yT)
                return

            # ============ Phase B: QKV -> DRAM scratch ============
            # kT_s is stored position-major (yT order); C maps seq->pos
            with tc.tile_pool(name="qkv_w", bufs=2) as wp, \
                 tc.tile_pool(name="qkv_ps", bufs=3, space="PSUM") as qps, \
                 tc.tile_pool(name="qkv_st", bufs=4) as stp:
                # --- Q (own rows, from yTo) ---
                # whole-matrix weight loads: few large descriptors instead of
                # per-feature-block strided gathers
                wq_all = wp.tile([P, c.DB, c.D], DT, tag="wq_all")
                nc.sync.dma_start(
                    wq_all, w_qkv[:, 0:c.D].rearrange("(o p) q -> p o q", p=P))
                for fo in range(c.DB):
                    for ch in range(c.SQ // NC):
                        ps = qps.tile([P, NC], F32, tag="qk_ps")
                        for f in range(c.DB):
                            nc.tensor.matmul(
                                ps, wq_all[:, f, fo * P:(fo + 1) * P],
                                yTo[:, f, ch * NC:(ch + 1) * NC],
                                start=(f == 0), stop=(f == c.DB - 1))
                        st = stp.tile([P, NC], DT, tag="qk_st")
                        nc.scalar.activation(
                            st, ps, mybir.ActivationFunctionType.Identity,
                            bias=cq_s[:, fo:fo + 1], scale=scale)
                        nc.sync.dma_start(
                            qT_s[fo * P:(fo + 1) * P, ch * NC:(ch + 1) * NC],
                            st)
                # --- K (full seq, from yT; store in seq order) ---
                wk_all = wp.tile([P, c.DB, c.D], DT, tag="wk_all")
                nc.sync.dma_start(
                    wk_all,
                    w_qkv[:, c.D:2 * c.D].rearrange("(o p) q -> p o q", p=P))
                for fo in range(c.DB):
                    for ch in range(c.S // NC):
                        ps = qps.tile([P, NC], F32, tag="qk_ps")
                        for f in range(c.DB):
                            nc.tensor.matmul(
                                ps, wk_all[:, f, fo * P:(fo + 1) * P],
                                yT[:, f, ch * NC:(ch + 1) * NC],
                                start=(f == 0), stop=(f == c.DB - 1))
                        st = stp.tile([P, NC], DT, tag="qk_st")
                        nc.scalar.activation(
                            st, ps, mybir.ActivationFunctionType.Identity,
                            bias=ck_b[:, fo:fo + 1], scale=1.0)
                        nc.sync.dma_start(
                            kT_s[fo * P:(fo + 1) * P, ch * NC:(ch + 1) * NC],
                            st)
                # --- V (full seq rows) ---
                for vc in range(c.D // NC):
                    wv = wp.tile([P, c.DB, NC], DT, tag="w_v")
                    wcol = w_qkv[:, 2 * c.D + vc * NC: 2 * c.D + (vc + 1) * NC]
                    nc.sync.dma_start(wv, wcol.rearrange("(o p) q -> p o q", p=P))
                    for pos in range(c.RB):
                        rb = pos_rb(pos)
                        ps = qps.tile([P, NC], F32, tag="v_ps")
                        for f in range(c.DB):
                            nc.tensor.matmul(
                                ps, yT[:, f, pos * P:(pos + 1) * P], wv[:, f, :],
                                start=(f == 0), stop=(f == c.DB - 1))
                        st = stp.tile([P, NC], DT, tag="v_st")
                        nc.scalar.copy(st, ps)
                        nc.sync.dma_start(
                            v_s[rb * P:(rb + 1) * P, vc * NC:(vc + 1) * NC], st)
            if stop_after == "B":
                dump_and_stop(yT)
                return

        # ===== Phase C: attention (St = K@Q^T; denominator via V|1) =====
        with tc.tile_pool(name="OT_pool", bufs=1) as OTp:
            OT = OTp.tile([P, c.DB, c.SQ], DT)
            ones_rb = OTp.tile([P, c.RB, 1], F32)
            nc.vector.memset(ones_rb, 1.0)
            with tc.tile_pool(name="at_v", bufs=1) as vsp, \
                 tc.tile_pool(name="at_in", bufs=3) as aip, \
                 tc.tile_pool(name="at_e", bufs=2) as ep, \
                 tc.tile_pool(name="at_sm", bufs=4) as smp, \
                 tc.tile_pool(name="at_sps", bufs=3, space="PSUM") as spsp, \
                 tc.tile_pool(name="at_ops", bufs=4, space="PSUM") as opsp:
                # one large-descriptor load of all of V; heads sliced on-chip
                v_sb = vsp.tile([P, c.RB, c.D], DT)
                nc.sync.dma_start(v_sb,
                                  v_s.rearrange("(rb p) d -> p rb d", p=P))
                for h in range(c.NH):
                    qTh = aip.tile([c.HD, c.SQ], DT, tag="qTh")
                    nc.sync.dma_start(qTh, qT_s[h * c.HD:(h + 1) * c.HD, :])
                    kTh = aip.tile([c.HD, c.S], DT, tag="kTh")
                    nc.sync.dma_start(kTh, kT_s[h * c.HD:(h + 1) * c.HD, :])
                    vh = aip.tile([P, c.RB, c.HD + 1], F32, tag="vh")
                    nc.vector.tensor_copy(
                        vh[:, :, :c.HD],
                        v_sb[:, :, h * c.HD:(h + 1) * c.HD])
                    nc.vector.tensor_copy(vh[:, :, c.HD:], ones_rb)
                    fo, fi = h // 2, (h % 2) * c.HD  # OT feature placement
                    dens = smp.tile([1, c.SQ], F32, tag="dens")
                    ops_tiles = []
                    for t in range(c.QB // 2):
                        j0, j1 = 2 * t, 2 * t + 1
                        nkb0 = 2 * j0 + 2
                        nkb1 = 2 * j1 + 2     # multiple of 4
                        E = ep.tile([P, nkb1, 2 * P], F32, tag="E",
                                    name=f"E_{t}")
                        ops = opsp.tile([c.HD + 1, 2, P], F32, tag="o_ps")
                        ops_tiles.append(ops)
                        opsf = ops.rearrange("d a b -> d (a b)")
                        for kb in range(nkb1):
                            kpos = xg_pos(kb)   # kT_s/yT are position-major
                            st = spsp.tile([P, 2 * P], F32, tag="st_ps")
                            nc.tensor.matmul(
                                st, kTh[:, kpos * P:(kpos + 1) * P],
                                qTh[:, j0 * P: j0 * P + 2 * P],
                                start=True, stop=True)
                            mi = kb - (nkb0 - 2)
                            if 0 <= mi < 4:
                                nc.vector.tensor_add(st, st, mask_sb[:, mi, :])
                            nc.scalar.activation(
                                E[:, kb, :], st,
                                mybir.ActivationFunctionType.Exp)
                            nc.tensor.matmul(
                                opsf, vh[:, kb, :], E[:, kb, :],
                                start=(kb == 0), stop=(kb == nkb1 - 1))
                        # stash both query-blocks' denominators [1, 2P]
                        nc.vector.tensor_copy(
                            dens[:, j0 * P: j0 * P + 2 * P],
                            ops.rearrange("d a b -> d (a b)")[c.HD:, :])
                    # one reciprocal + one gpsimd broadcast per head
                    rcpd = smp.tile([1, c.SQ], F32, tag="rcpd")
                    nc.vector.reciprocal(rcpd, dens)
                    rbig = smp.tile([c.HD, c.SQ], F32, tag="rbig")
                    nc.gpsimd.partition_broadcast(rbig, rcpd)
                    for t in range(c.QB // 2):
                        ops = ops_tiles[t]
                        for a, j in ((0, 2 * t), (1, 2 * t + 1)):
                            nc.vector.tensor_mul(
                                OT[fi:fi + c.HD, fo, j * P:(j + 1) * P],
                                ops[:c.HD, a, :],
                                rbig[:, j * P:(j + 1) * P])
            if stop_after == "C":
                dump_and_stop(OT)
                return

            # ====== Phase D1: proj + residual + LN2 + transpose ======
            with tc.tile_pool(name="y2T_pool", bufs=1) as y2Tp:
                y2T = y2Tp.tile([P, c.DB, c.SQ], DT)
                out_acc = y2Tp.tile([P, c.QB, c.D], F32)
                with tc.tile_pool(name="pr_w", bufs=1) as pwp, \
                     tc.tile_pool(name="pr_work", bufs=3) as prw, \
                     tc.tile_pool(name="pr_ps", bufs=3, space="PSUM") as prps, \
                     tc.tile_pool(name="pr_tps", bufs=3, space="PSUM") as prtps:
                    wproj_sb = pwp.tile([P, c.DB, c.D], DT)
                    nc.sync.dma_start(
                        wproj_sb, w_proj.rearrange("(o p) q -> p o q", p=P))
                    xsh_b = xsh.rearrange("(rb p) d -> rb p d", p=P)
                    for rq in range(c.QB):
                        # delta accumulates everything except the x residual;
                        # the host adds x back in f32 (out = x + delta)
                        dl_t = prw.tile([P, c.D], F32, tag="dl")
                        x2_t = prw.tile([P, c.D], F32, tag="x2")
                        xo = prw.tile([P, c.D], DT, tag="xo")
                        nc.sync.dma_start(xo, xsh_b[rq])
                        for fc in range(c.D // NC):
                            ps = prps.tile([P, NC], F32, tag="pr_ps")
                            for hp in range(c.DB):
                                nc.tensor.matmul(
                                    ps, OT[:, hp, rq * P:(rq + 1) * P],
                                    wproj_sb[:, hp, fc * NC:(fc + 1) * NC],
                                    start=(hp == 0), stop=(hp == c.DB - 1))
                            dsl = dl_t[:, fc * NC:(fc + 1) * NC]
                            nc.vector.tensor_add(
                                dsl, ps, beff_b[:, fc * NC:(fc + 1) * NC])
                            nc.vector.tensor_add(
                                x2_t[:, fc * NC:(fc + 1) * NC], dsl,
                                xo[:, fc * NC:(fc + 1) * NC])
                        nc.vector.tensor_add(out_acc[:, rq, :], dl_t, b2_b)
                        y2_t = prw.tile([P, c.D], DT, tag="y2")
                        layernorm(prw, x2_t, y2_t)
                        for f in range(c.DB):
                            pt = prtps.tile([P, P], DT, tag="tp2")
                            nc.tensor.transpose(
                                pt, y2_t[:, f * P:(f + 1) * P], ident)
                            nc.vector.tensor_copy(
                                y2T[:, f, rq * P:(rq + 1) * P], pt)

                if stop_after == "D1":
                    dump_and_stop(y2T)
                    return

                # ===== Phase D2: MLP, PSUM-chained w2 accumulation =====
                NRB = c.SQ // P
                NCH = c.SQ // NC
                GH = 8                      # hidden blocks per group
                with tc.tile_pool(name="mlp_w", bufs=2) as mwp, \
                     tc.tile_pool(name="mlp_h", bufs=2) as mhp, \
                     tc.tile_pool(name="mlp_ps", bufs=3, space="PSUM") as mps, \
                     tc.tile_pool(name="m2_ps", bufs=4, space="PSUM") as m2ps:
                    for gi in range(c.HB // GH):
                        hT8 = mhp.tile([P, GH, c.SQ], DT, tag="hT8")
                        w2r8 = mhp.tile([P, GH, c.D], DT, tag="w2r8")
                        # one load per group: GH*P w1 columns, 2KB descriptors
                        w1g = mwp.tile([P, c.DB, GH * P], DT, tag="w1g")
                        nc.sync.dma_start(
                            w1g, w1[:, gi * GH * P:(gi + 1) * GH * P]
                            .rearrange("(o p) q -> p o q", p=P))
                        for hl in range(GH):
                            hb = gi * GH + hl
                            nc.sync.dma_start(w2r8[:, hl, :], w2_block(hb))
                            for chq in range(NCH):
                                ps = mps.tile([P, NC], F32, tag="h_ps")
                                for f in range(c.DB):
                                    nc.tensor.matmul(
                                        ps,
                                        w1g[:, f, hl * P:(hl + 1) * P],
                                        y2T[:, f, chq * NC:(chq + 1) * NC],
                                        start=(f == 0), stop=(f == c.DB - 1))
                                nc.scalar.activation(
                                    hT8[:, hl, chq * NC:(chq + 1) * NC], ps,
                                    mybir.ActivationFunctionType.Gelu_apprx_tanh,
                                    bias=b1c[:, hb:hb + 1], scale=1.0)
                        for rb in range(NRB):
                            for fc in range(c.D // NC):
                                ps2 = m2ps.tile([P, NC], F32, tag="m2_ps")
                                for hl in range(GH):
                                    nc.tensor.matmul(
                                        ps2,
                                        hT8[:, hl, rb * P:(rb + 1) * P],
                                        w2r8[:, hl, fc * NC:(fc + 1) * NC],
                                        start=(hl == 0), stop=(hl == GH - 1))
                                sl = out_acc[:, rb, fc * NC:(fc + 1) * NC]
                                nc.vector.tensor_add(sl, sl, ps2)
                    ob3 = out.rearrange("(rb p) d -> rb p d", p=P)
                    for rb in range(NRB):
                        # SWDGE dma casts f32 accumulator -> bf16 output
                        nc.gpsimd.dma_start(ob3[rb], out_acc[:, rb, :])

# =================== host side ===================


def prepare_shared(inputs, cfg):
    """Host prep common to all cores: fold LN scales/biases, cast to bf16."""
    import ml_dtypes
    BF = ml_dtypes.bfloat16
    c = cfg
    f32 = lambda k: np.asarray(inputs[k], np.float32)
    w_qkv, w_proj, w1, w2 = f32("w_qkv"), f32("w_proj"), f32("w1"), f32("w2")
    ln1_s, ln1_b = f32("ln1_scale"), f32("ln1_bias")
    ln2_s, ln2_b = f32("ln2_scale"), f32("ln2_bias")
    b_proj, b1, b2 = f32("b_proj"), f32("b1"), f32("b2")
    attn_scale = np.float32(1.0 / np.sqrt(c.HD))

    # fold LN scales into weights; LN biases into matmul biases
    wq_eff = (ln1_s[:, None] * w_qkv).astype(BF)     # [D, 3D]
    c3 = ln1_b @ w_qkv                               # [3D]
    cq, ck, cv = c3[:c.D], c3[c.D:2 * c.D], c3[2 * c.D:]
    b_eff = cv @ w_proj + b_proj                     # [D]
    w1_eff = (ln2_s[:, None] * w1).astype(BF)        # [D, HID]
    b1_eff = b1 + ln2_b @ w1                         # [HID]
    w_proj_bf = w_proj.astype(BF)
    w2_bf = w2.astype(BF)
    x_bf = np.asarray(inputs["x"], np.float32).astype(BF)  # [B, S, D]

    # causal-mask thresholds: mask[:, i, f] = (f < thr[:, i]) * -1e9.
    # Applied at key blocks nkb0-2 .. nkb0+1 of each query pair (see phase C).
    k = np.arange(P, dtype=np.float32)
    thrs = {
        0: np.stack([k, k * 0 + 128, 128 + k, k * 0 + 512], axis=1),
        1: np.stack([k * 0 - 1, k, k * 0 + 128, 128 + k], axis=1),
    }

    po = lambda v: np.ascontiguousarray(v.reshape(-1, P).T)  # [(o p)] -> [p, o]
    vcols = {}
    for p in range(2):
        vcol = np.empty((P, 52), np.float32)
        vcol[:, 0:8] = po(cq * attn_scale)
        vcol[:, 8:16] = po(ck)
        vcol[:, 16:48] = po(b1_eff)
        vcol[:, 48:52] = thrs[p]
        vcols[p] = vcol
    vrow = np.concatenate([b_eff, b2]).astype(np.float32)
    return dict(wq_eff=wq_eff, w_proj=w_proj_bf, w1_eff=w1_eff, w2=w2_bf,
                x=x_bf, vcols=vcols, vrow=vrow, BF=BF)


def make_core_inputs(shared, cfg, b, p, rank):
    """Per-core input map for core (batch b, parity p) = rank 2b+p."""
    c = cfg
    s = shared
    r = rank
    wblob = np.empty((P, WCOLS), s["BF"])
    rows = slice(r * P, (r + 1) * P)
    wblob[:, WQKV0:WQKV0 + 3 * c.D] = s["wq_eff"][rows]
    wblob[:, WPROJ0:WPROJ0 + c.D] = s["w_proj"][rows]
    wblob[:, W10:W10 + c.HID] = s["w1_eff"][rows]
    w2r = s["w2"][r * 4 * P:(r + 1) * 4 * P]          # [512, D]
    wblob[:, W20:] = w2r.reshape(4, P, c.D).transpose(1, 0, 2).reshape(P, 4 * c.D)

    xsh = s["x"][b].reshape(c.RB, P, c.D)[p::2].reshape(c.SQ, c.D)

    # byte-plane split: [rows, N] bf16 -> [rows, 2N] u8 (high plane | low)
    def plane16(a):
        v = np.ascontiguousarray(a).view(np.uint16)
        return np.concatenate(
            [(v >> 8).astype(np.uint8), (v & 255).astype(np.uint8)], axis=-1)

    return {
        "wblob": plane16(wblob),
        "xsh": plane16(xsh),
        "vcol": s["vcols"][p],
        "vrow": s["vrow"],
    }


_CACHE = {}


def get_nc(cfg, reps=1, stop_after=None, enable_asserts=False, use_f32r=None):
    key = (cfg.S, cfg.D, cfg.NH, cfg.HID, cfg.NC, reps, stop_after)
    if key not in _CACHE:
        nc = bacc.Bacc("TRN2", target_bir_lowering=False, debug=False,
                       enable_asserts=enable_asserts, num_devices=8)
        with tile.TileContext(nc) as tc:
            build(nc, tc, cfg, reps=reps, stop_after=stop_after)
        nc.compile()
        _CACHE[key] = nc
    return _CACHE[key]


USE_F32R = False  # kept for test.py compat (ignored; v2 is bf16)


def kernel(**inputs):
    from concourse.bass_utils import run_bass_kernel_spmd
    cfg = Cfg()
    nc = get_nc(cfg)
    shared = prepare_shared(inputs, cfg)
    in_maps = [make_core_inputs(shared, cfg, i // 2, i % 2, i) for i in range(8)]
    res = run_bass_kernel_spmd(nc, in_maps, list(range(8))).results
    B = 4
    # device returns delta = out - x (bf16); add the f32 residual on host
    outf = np.array(np.asarray(inputs["x"]), np.float32, copy=True)
    ob = outf.reshape(B, cfg.RB, P, cfg.D)
    for i in range(8):
        b, p = i // 2, i % 2
        ob[b, p::2] += np.asarray(
            res[i]["out"], np.float32).reshape(cfg.QB, P, cfg.D)
    return outf
